# revision 39
# baseline (speedup 1.0000x reference)
"""Causal self-attention (B=4, S=2048, D=1024, H=16, HD=64) on 8 trn2 cores.

Sharding: core c handles batch b = c//2 and head-group g = c%2 (8 heads).
Each core computes its 8 heads' attention plus the partial output
projection over its d-slice; the host adds the two partial y's per batch.

Device layout is fully transposed ([feature, seq]) so every matmul
contraction lands on the partition dim with no on-device transposes:
  qkvT = wqkvT^T @ xT        (bf16 inputs, fp32 psum, e on partitions)
  scoresT[s_k, s_q] = kT^T @ qT                (float32r, causal-trimmed)
  pT = exp(scoresT/8)        (ACT, bf16 out; triangular mask on boundary)
  out_aug[128, s_q] = v_aug^T @ pT   (bf16; rows 64-127 = ones block ->
                                      64 replicated softmax denominators)
  yT = wprojT^T @ (outT / denom)               (float32r)
QKV(n=j+1) and proj(j-1) matmul chains are interleaved into attention
column j so the PE never idles long enough for HAM to re-throttle.
"""

from contextlib import ExitStack

import ml_dtypes
import numpy as np

import concourse.bacc as bacc
import concourse.mybir as mybir
import concourse.tile as tile
from concourse._compat import with_exitstack
from concourse.bass import ds, ts  # noqa: E402
from concourse.bass_utils import run_bass_kernel_spmd
from concourse.masks import make_upper_triangular

B, S, D = 4, 2048, 1024
H, HD = 16, 64
P = 128
GH = 8            # heads per core
DS = GH * HD      # 512, d-slice per core
E = 3 * DS        # 1536 qkv features per core
KD = D // P       # 8 contraction subtiles for qkv
KP = DS // P      # 4 contraction subtiles for proj
NJ = S // 512     # 4 s_q tiles of 512
NST = S // P      # 16 s_k tiles of 128
F32 = mybir.dt.float32
F32R = mybir.dt.float32r
BF16 = mybir.dt.bfloat16
EXP = mybir.ActivationFunctionType.Exp


@with_exitstack
def _emit(ctx: ExitStack, tc: tile.TileContext, xT, wqkvT, wprojT, yT):
    nc = tc.nc

    xT_t = xT.rearrange("(ko ki) s -> ki ko s", ki=P)      # [128, 8, 2048]
    wq_t = wqkvT.rearrange("(ko ki) e -> ki ko e", ki=P)   # [128, 8, 1536]
    wp_t = wprojT.rearrange("(ko ki) e -> ki ko e", ki=P)  # [128, 4, 1024]
    yT_t = yT.rearrange("(mo mi) s -> mi mo s", mi=P)      # [128, 8, 2048]

    const = ctx.enter_context(tc.tile_pool(name="const", bufs=1))
    qk_pool = ctx.enter_context(tc.tile_pool(name="qkp", bufs=1))
    big = ctx.enter_context(tc.tile_pool(name="big", bufs=1))
    pt_pool = ctx.enter_context(tc.tile_pool(name="ptp", bufs=8))
    xin = ctx.enter_context(tc.tile_pool(name="xin", bufs=2))
    ot_pool = ctx.enter_context(tc.tile_pool(name="otp", bufs=3))
    sm = ctx.enter_context(tc.tile_pool(name="sm", bufs=2))
    yout = ctx.enter_context(tc.tile_pool(name="yo", bufs=3))
    ps_sc = ctx.enter_context(tc.tile_pool(name="ps_sc", bufs=2, space="PSUM"))
    ps_pv = ctx.enter_context(tc.tile_pool(name="ps_pv", bufs=2, space="PSUM"))

    # per-k-subtile weight tiles so the first qkv chain starts as soon as
    # its chunk lands, instead of waiting for the whole 3MB wqkv DMA;
    # q|k columns and v columns are separate tiles: the v halves are only
    # needed once pair-0 PV starts, so they queue after all q|k chunks
    wqks = [const.tile([P, 2 * DS], BF16, name=f"wqk{k}") for k in range(KD)]
    wvs = [const.tile([P, DS], BF16, name=f"wv{k}") for k in range(KD)]
    wp = const.tile([P, KP, D], BF16)  # DMA deferred to the prelude
    mask = const.tile([P, P], BF16)
    make_upper_triangular(nc, mask[:], val=1.0, diag=True)

    # qkT: e-tiles 0-3 = q head pairs, 4-7 = k head pairs; [e_in, tile, s]
    qk = qk_pool.tile([P, 8, S], BF16)
    # v natural layout + 64-wide ones block per head: [s_in, s_tile, head, 128]
    # PV with this lhsT gives psum rows 0-63 = out, 64-127 = denom copies.
    vaug = big.tile([P, NST, GH, 2 * HD], BF16)
    ones = const.tile([P, 1], F32)
    nc.vector.memset(ones[:], 1.0)
    nc.gpsimd.tensor_copy(vaug[:, :, :, HD:], ones.to_broadcast((P, NST, GH, HD)))

    xts = [None] * NJ

    def load_x(n):
        # per-k-subtile chunks: chain k is gated only on its own chunk
        xt = [xin.tile([P, 512], BF16, tag=f"xt{k}", name="xt") for k in range(KD)]
        for k in range(KD):
            eng = nc.sync if k % 2 == 0 else nc.scalar
            eng.dma_start(xt[k][:], xT_t[:, k, ts(n, 512)])
        xts[n] = xt

    def qkv_qk_chain(n, m):
        ps = ps_sc.tile([P, 1024], F32, tag="sc", name="ps")[:, 0:512]
        for k in range(KD):
            nc.tensor.matmul(ps[:], wqks[k][:, ts(m, P)], xts[n][k][:],
                             start=(k == 0), stop=(k == KD - 1))
        # blocks 0-2 run during cols 0-1 where the DVE queue is packed with
        # epilogues but ACT still has exp headroom; block 3 runs in col 2
        # where ACT is near-saturated, so its copies go to the DVE
        if n <= 2:
            nc.scalar.copy(qk[:, m, ts(n, 512)], ps[:])
        else:
            nc.vector.tensor_copy(qk[:, m, ts(n, 512)], ps[:])

    def qkv_v_chain(n, ss):
        st = n * 4 + ss
        ps = ps_sc.tile([P, 1024], F32, tag="sc", name="ps")[:, 0:512]
        for k in range(KD):
            nc.tensor.matmul(ps[:], xts[n][k][:, ts(ss, P)], wvs[k][:],
                             start=(k == 0), stop=(k == KD - 1))
        psv = ps.rearrange("p (h d) -> p h d", h=GH)
        if n <= 2:
            nc.scalar.copy(vaug[:, st, :, 0:HD], psv)
        else:
            nc.vector.tensor_copy(vaug[:, st, :, 0:HD], psv)

    outTs = [None] * NJ

    def attn_pair(l, j, pacer=None, tail=False):
        outT = outTs[j]
        imax = 4 * (j + 1)
        pv = ps_pv.tile([P, 1024], F32, tag="pv", name="pv")
        pts = {}

        def emit_scores(i):
            t = i - 4 * j  # >=0 -> diagonal boundary tile
            off = 128 * t if t > 0 else 0
            sc = ps_sc.tile([P, 1024], F32, tag="sc", name="sc")
            scv = sc.rearrange("p (u f) -> p u f", u=2)
            nc.tensor.matmul(sc[:, off:512], qk[0:64, 4 + l, ts(i, P)],
                             qk[0:64, l, ds(j * 512 + off, 512 - off)],
                             start=True, stop=True)
            nc.tensor.matmul(sc[:, 512 + off:1024], qk[64:128, 4 + l, ts(i, P)],
                             qk[64:128, l, ds(j * 512 + off, 512 - off)],
                             start=True, stop=True)
            pt = pt_pool.tile([P, 1024], BF16, tag="pt", name="pt")
            ptv = pt.rearrange("p (u f) -> p u f", u=2)
            nc.scalar.activation(ptv[:, :, off:512], scv[:, :, off:512],
                                 EXP, scale=0.125)
            if t >= 0:  # causal mask on the boundary 128-col block (Pool eng)
                nc.gpsimd.tensor_tensor(
                    ptv[:, :, off:off + P], ptv[:, :, off:off + P],
                    mask[:, None, :].to_broadcast((P, 2, P)),
                    mybir.AluOpType.mult)
            pts[i] = (pt, off)

        # software-pipelined 2 deep: scores/EXP for i+2 are emitted BEFORE
        # the PV matmuls of i, so EXP and the boundary mask (ACT + Pool
        # hops) have two full steps to finish before PV consumes pt
        for w in range(min(2, imax)):
            emit_scores(w)
        for i in range(imax):
            if pacer is not None:
                pacer.tick()
            if i + 2 < imax:
                emit_scores(i + 2)
            pt, off = pts.pop(i)
            nc.tensor.matmul(pv[:, off:512], vaug[:, i, 2 * l, :],
                             pt[:, off:512],
                             start=(i == 0), stop=(i == imax - 1))
            nc.tensor.matmul(pv[:, 512 + off:1024], vaug[:, i, 2 * l + 1, :],
                             pt[:, 512 + off:1024],
                             start=(i == 0), stop=(i == imax - 1))
        # epilogue deferred into the next pair's tick stream so these DVE ops
        # don't queue ahead of the qkv/proj psum-slot CASTs the PE waits on.
        # partition crossing (psum rows 64:128 -> sbuf rows 0:64) must be a
        # copy: custom-DVE/TT ops misread partition-shifted operands.
        if tail:
            # final pair feeds the tail proj directly: dens on the (idle)
            # ACT engine, DVE ops interleaved across the two heads so the
            # dependency chains pipeline instead of serializing
            def epi_all():
                dens = []
                for hh in (0, 1):
                    half = pv[:, 512 * hh:512 * (hh + 1)]
                    den = sm.tile([HD, 512], F32, tag="den", name="den")
                    nc.scalar.copy(den[:], half[HD:2 * HD, :])
                    dens.append((den, half))
                recs = []
                for den, half in dens:
                    rec = sm.tile([HD, 512], F32, tag="rec", name="rec")
                    nc.vector.reciprocal_approx_fast(rec[:], den[:])
                    recs.append(rec)
                for hh in (0, 1):
                    nc.vector.tensor_tensor(outT[hh * HD:(hh + 1) * HD, l, :],
                                            dens[hh][1][0:HD, :], recs[hh][:],
                                            mybir.AluOpType.mult)
            return [epi_all]

        def epi(hh):
            half = pv[:, 512 * hh:512 * (hh + 1)]
            den = sm.tile([HD, 512], F32, tag="den", name="den")
            nc.vector.tensor_copy(den[:], half[HD:2 * HD, :])
            rec = sm.tile([HD, 512], F32, tag="rec", name="rec")
            nc.vector.reciprocal_approx_fast(rec[:], den[:])
            nc.vector.tensor_tensor(outT[hh * HD:(hh + 1) * HD, l, :],
                                    half[0:HD, :], rec[:],
                                    mybir.AluOpType.mult)
        return [lambda hh=hh: epi(hh) for hh in (0, 1)]

    def proj_col_chain(j, m):
        ps = ps_sc.tile([P, 1024], F32, tag="sc", name="ps")[:, 0:512]
        for k in range(KP):
            nc.tensor.matmul(ps[:], wp[:, k, ts(m, P)], outTs[j][:, k, :],
                             start=(k == 0), stop=(k == KP - 1))
        yt = yout.tile([P, 512], BF16, tag="yt", name="yt")
        nc.vector.tensor_copy(yt[:], ps[:])
        nc.sync.dma_start(yT_t[:, m, ts(j, 512)], yt[:])

    def proj(j):
        # tail-only: yt copies on ACT (idle by then) so the DVE only has to
        # keep up with the psum-slot CASTs between chains
        for m in range(8):
            ps = ps_sc.tile([P, 1024], F32, tag="sc", name="ps")[:, 0:512]
            for k in range(KP):
                nc.tensor.matmul(ps[:], wp[:, k, ts(m, P)], outTs[j][:, k, :],
                                 start=(k == 0), stop=(k == KP - 1))
            yt = yout.tile([P, 512], BF16, tag="yt", name="yt")
            nc.scalar.copy(yt[:], ps[:])
            nc.sync.dma_start(yT_t[:, m, ts(j, 512)], yt[:])

    class Pacer:
        # Bresenham-paced emission of filler matmul chains between
        # attention iterations, to keep the PE dense (HAM stays warm).
        # Urgent thunks (deferred softmax normalizes) fire one per tick
        # ahead of the paced stream so DVE recips interleave with, not
        # ahead of, the next pair's mask multiplies.
        def __init__(self, thunks, total_ticks):
            self.thunks = list(thunks)
            self.total = max(1, total_ticks)
            self.ticks = 0
            self.fired = 0
            self.urgent = []

        def inject(self, thunks):
            self.urgent.extend(thunks)

        def tick(self):
            self.ticks += 1
            # paced chain FIRST so its slot-critical psum->sbuf CAST lands
            # ahead of the epilogue ops in the DVE queue; urgent epilogue
            # fires after, in addition (it's DVE/ACT work, not PE work)
            while (self.fired < len(self.thunks)
                   and self.fired * self.total < self.ticks * len(self.thunks)):
                self.thunks[self.fired]()
                self.fired += 1
            if self.urgent:
                self.urgent.pop(0)()

        def flush(self):
            for t in self.urgent:
                t()
            self.urgent = []
            while self.fired < len(self.thunks):
                self.thunks[self.fired]()
                self.fired += 1

    # prelude: wqkv-qk-k / x0-k chunk DMAs interleaved pairwise on the two
    # HWDGE queues so qkv chain k can start as soon as its pair lands; the
    # v halves and wproj queue after the startup-critical chunks
    xt0 = [xin.tile([P, 512], BF16, tag=f"xt{k}", name="xt") for k in range(KD)]
    for k in range(KD):
        (nc.sync if k % 2 == 0 else nc.scalar).dma_start(
            wqks[k][:], wq_t[:, k, 0:2 * DS])
        (nc.scalar if k % 2 == 0 else nc.sync).dma_start(
            xt0[k][:], xT_t[:, k, ts(0, 512)])
    for k in range(KD):
        (nc.sync if k % 2 == 0 else nc.scalar).dma_start(
            wvs[k][:], wq_t[:, k, 2 * DS:3 * DS])
    xts[0] = xt0
    nc.sync.dma_start(wp[:], wp_t)
    # only the chains pair 0 needs run unpaced; the rest go through the
    # column-0 pacer so pair-0 attention overlaps the startup DMA tail
    for m in (0, 4):
        qkv_qk_chain(0, m)
    for ss in range(4):
        qkv_v_chain(0, ss)

    # attention column j; QKV(j+1) and proj(j-1) chains paced into the
    # attention iteration stream
    for j in range(NJ):
        outTs[j] = ot_pool.tile([P, KP, 512], BF16, tag="outT", name="outT")
        if j + 1 < NJ:
            load_x(j + 1)
        thunks = []
        if j == 0:
            # block-0 chains for pairs 1-3, in the order the pairs need them
            for m in (1, 5, 2, 6, 3, 7):
                thunks.append(lambda m=m: qkv_qk_chain(0, m))
        if j + 1 < NJ:
            for m in range(8):
                thunks.append(lambda n=j + 1, m=m: qkv_qk_chain(n, m))
            for ss in range(4):
                thunks.append(lambda n=j + 1, ss=ss: qkv_v_chain(n, ss))
        # proj filler weighted toward the late (long-attention) columns so
        # the PE keeps pace with the ACT exp cadence everywhere
        for jj in ({2: [0], 3: [1, 2]}.get(j, [])):
            for m in range(8):
                thunks.append(lambda jj=jj, m=m: proj_col_chain(jj, m))
        # last column: reserve ticks so a few proj chains remain at flush
        # to fill the PE while the final pair's epilogue completes
        pacer = Pacer(thunks, 4 * 4 * (j + 1) + (6 if j == NJ - 1 else 0))
        for l in range(4):
            pacer.inject(attn_pair(l, j, pacer,
                                   tail=(l == 3 and j == NJ - 1)))
        pacer.flush()
    proj(NJ - 1)


_NC = None


def build_nc():
    global _NC
    if _NC is not None:
        return _NC
    nc = bacc.Bacc("TRN2", target_bir_lowering=False, debug=False)
    xT = nc.dram_tensor("xT", [D, S], BF16, kind="ExternalInput")
    wqkvT = nc.dram_tensor("wqkvT", [D, E], BF16, kind="ExternalInput")
    wprojT = nc.dram_tensor("wprojT", [DS, D], BF16, kind="ExternalInput")
    yT = nc.dram_tensor("yT", [D, S], BF16, kind="ExternalOutput")
    with tile.TileContext(nc) as tc:
        _emit(tc, xT.ap(), wqkvT.ap(), wprojT.ap(), yT.ap())
    nc.compile()
    _NC = nc
    return nc


def make_in_maps(x, w_attn, w_proj):
    x = np.ascontiguousarray(np.asarray(x, dtype=np.float32))
    w_attn = np.asarray(w_attn, dtype=np.float32)
    w_proj = np.asarray(w_proj, dtype=np.float32)
    in_maps = []
    for c in range(8):
        b, g = divmod(c, 2)
        rows = slice(g * DS, (g + 1) * DS)
        wqkv_c = np.concatenate(
            [w_attn[0 * D:1 * D][rows], w_attn[1 * D:2 * D][rows],
             w_attn[2 * D:3 * D][rows]], axis=0)          # [1536, 1024]
        in_maps.append({
            "xT": np.ascontiguousarray(x[b].T).astype(ml_dtypes.bfloat16),
            "wqkvT": np.ascontiguousarray(wqkv_c.T).astype(ml_dtypes.bfloat16),
            "wprojT": np.ascontiguousarray(w_proj[:, rows].T).astype(
                ml_dtypes.bfloat16),               # [512, 1024]
        })
    return in_maps


def gather(results):
    y = np.empty((B, S, D), dtype=np.float32)
    for b in range(B):
        yT = (results[2 * b]["yT"].astype(np.float32)
              + results[2 * b + 1]["yT"].astype(np.float32))
        y[b] = yT.T
    return y


def run(x, w_attn, w_proj, trace=False, tmpdir=None):
    nc = build_nc()
    in_maps = make_in_maps(x, w_attn, w_proj)
    res = run_bass_kernel_spmd(nc, in_maps, list(range(8)),
                               trace=trace, tmpdir=tmpdir)
    return gather(res.results), res


def kernel(x, w_attn, w_proj):
    y, _ = run(x, w_attn, w_proj)
    return y

